# revision 70
# baseline (speedup 1.0000x reference)
"""Trainium2 Bass kernel for BSplineBasis (degree-3, 64 uniform-ish knots).

Math: for each normalized point xn and each of 60 basis elements i the
reference evaluates a piecewise cubic (de Boor with clamped interval index).
With simple inner knots this is exactly the truncated-power form

    out[n, i] = sum_q A'[q,i] * y^q  +  sum_m J[m,i] * relu(y - kap'_m)^3

with y = xn - 0.5, 56 inner-knot features and a banded jump table J.

Device pipeline per 980-point iteration (two 490-point tile-pairs, tiles
(pb, pb+64) block-diagonal so fp32r matmuls keep start_partition 0):
  MM1 (K=6, fp32r): cube polynomials + y-power pass-throughs from rows
      [y0, y0^2, y0^3], where y0 = x/16 is a STATIC normalization computed
      before the global min/max is known; the data-dependent basis change
      y = a*y0 + b' is folded into MM1's weights on-device by a Horner chain over
      host-precomputed shifted weight blocks (and a 3-term chain for bias).
  relu (DVE tensor_scalar or ACT activation(Relu), per-partition bias):
      every stack row is a relu: cube rows natively, the power rows via
      positive shifts (1, y+0.5=xn>=0, y^2>=0, y^3+0.125>=0) with the
      shifts absorbed into w2's constant-row column.
  MM2 (K=120, fp32r): out = blockdiag(w2s, w2s).T @ stack -> PSUM
  evict (the other of DVE/ACT): PSUM -> SBUF bf16 staging
  DMA: staging -> out_t [120, 31360] bf16

The global min/max is computed redundantly per core from the full x
(5 column chunks: DVE free-axis min reduces, GPSIMD XYZWC max reduces),
overlapped with the input DMA. Each core's xf is host-rotated so its own
shard is chunk 0, letting the static y0 power table (all on DVE,
ahead of the chunk reduces) and the 3 merged layout-conversion DMAs (xp -> MM1-ready xr
rows, partition-strided destinations) run before min/max lands. The main
loop is software-pipelined (MM1 of iteration i+1 issues before MM2 of
iteration i) so the in-order tensor engine never parks behind a
sem-waiting matmul; the 64 PSUM->SBUF passes are split relu->ACT /
evict->DVE with a few both-on-ACT iterations to balance engine totals.
"""
import os
import sys

import numpy as np

if "/opt/trn_rl_repo" not in sys.path:
    sys.path.insert(0, "/opt/trn_rl_repo")

DEGREE = 3
NUM_KNOTS = 64
NB = NUM_KNOTS - DEGREE - 1          # 60 basis elements
N_POINTS = 500_000
N_CORES = 8
SHARD = N_POINTS // N_CORES          # 62500
TILE_W = 490                          # points per tile (= one partition row)
N_TILES = 128                         # 128 * 490 = 62720 >= SHARD
SHARD_PAD = N_TILES * TILE_W          # 62720
PAIRS = N_TILES // 2                  # 64 pairs (t, t+64)
NITER = PAIRS // 2                    # 32 iterations x 2 pairs
NF = 56                               # truncated-power features
XF_COLS = N_CORES * TILE_W            # 3920


# ----------------------------------------------------------------- host math
def _piece_poly_coeffs(knots, i, ell):
    """Monomial coeffs (len 4) of the de Boor piece for element i, interval
    ell in [3,6] — replicates the reference recursion, fit exactly in f64."""
    k = DEGREE
    seg = knots[i:i + k + 2]
    T = np.concatenate([np.full(k, seg[0] - 1.0), seg, np.full(k, seg[-1] + 1.0)])

    def eval_at(x):
        res = [np.float64(1.0)] + [np.float64(0.0)] * k
        for j in range(1, k + 1):
            hh = list(res[:j])
            res[0] = np.float64(0.0)
            for n in range(1, j + 1):
                tb, ta = T[ell + n], T[ell + n - j]
                den = tb - ta
                w = 0.0 if den == 0 else hh[n - 1] / den
                res[n - 1] = res[n - 1] + w * (tb - x)
                res[n] = w * (x - ta)
        return res[2 * k - ell]

    xs = np.linspace(-0.3, 1.3, 5)
    V = np.vander(xs, 4, increasing=True)
    return np.linalg.lstsq(V, np.array([eval_at(x) for x in xs]), rcond=None)[0]


def build_tables(knots):
    """A [4,60] (xn basis), J [56,60], row1 [60] for the truncated-power form."""
    knots = np.asarray(knots, np.float64)
    P = [[_piece_poly_coeffs(knots, i, p + 3) for p in range(4)] for i in range(NB)]

    def p_of(s, i):
        return int(np.clip(s - i - 1, 0, 3))

    A = np.zeros((4, NB))
    for i in range(NB):
        A[:, i] = P[i][p_of(4, i)]

    ms = list(range(4, 60))
    J = np.zeros((len(ms), NB))
    for f, m in enumerate(ms):
        for i in range(NB):
            pb, pa = p_of(m, i), p_of(m + 1, i)
            if pa != pb:
                J[f, i] = (P[i][pa] - P[i][pb])[3]

    # reference row at xn == 1.0 exactly (searchsorted jumps to s=64 there);
    # patched on the host for the (rare) argmax hits
    row1 = np.array([np.polyval(P[i][3][::-1], 1.0) for i in range(NB)])
    return A, J, row1


CENTER = 0.5  # the y-basis: y = xn - 0.5 (tames monomial cancellation)


def _shift_poly(c, h):
    """coeffs of p(y + h) given coeffs c of p(x), low->high, exact in f64."""
    from math import comb
    out = np.zeros_like(c)
    for q in range(4):
        for r in range(q + 1):
            out[r] += c[q] * comb(q, r) * h ** (q - r)
    return out


def _make_const_arrays(knots):
    A, J, row1 = build_tables(knots)
    # A in the y basis
    Ay = np.stack([_shift_poly(A[:, i], CENTER) for i in range(NB)], 1)  # [4,60]
    kap = np.asarray(knots, np.float64)[4:60] - CENTER                   # kappa'

    # static MM1 weights, y basis: rows q = weight on y^{q+1}; col layout:
    # 0..55 cube features, 56 const-1 (bias only), 57..59 power rows
    c3 = np.zeros((3, NB))
    c3[0, :NF] = 3.0 * kap**2
    c3[1, :NF] = -3.0 * kap
    c3[2, :NF] = 1.0
    for q in (1, 2, 3):
        c3[q - 1, NF + q] = 1.0
    cu_b = np.zeros((6, 2 * NB), np.float32)
    cu_b[0:3, :NB] = c3
    cu_b[3:6, NB:2 * NB] = c3
    # Horner composition: cu_eff[q'] = a^{q'+1} * (cu_b[q'] +
    # b' * (q'+2) * cu_b[q'+1] + b'^2 * binom(q'+3,q'+1) * cu_b[q'+2])
    cu_s = np.zeros((6, 3 * 2 * NB + 3), np.float32)
    cu_s[:, 0:2 * NB] = cu_b
    for h in (0, 1):
        for qp in (0, 1):
            cu_s[3 * h + qp, 2 * NB:4 * NB][h * NB:(h + 1) * NB] = \
                (qp + 2) * cu_b[3 * h + qp + 1, h * NB:(h + 1) * NB]
        cu_s[3 * h + 0, 4 * NB:6 * NB][h * NB:(h + 1) * NB] = \
            3.0 * cu_b[3 * h + 2, h * NB:(h + 1) * NB]
        for qp in range(3):
            cu_s[3 * h + qp, 6 * NB + qp] = 1.0   # E masks for apow
    # per-partition transpose of cu_b for the bias composition
    cusT = np.zeros((2 * NB, 3), np.float32)
    for h in (0, 1):
        cusT[h * NB:(h + 1) * NB, :] = cu_b[3 * h:3 * h + 3,
                                            h * NB:(h + 1) * NB].T
    # static bias, y basis (relu shifts on the power rows)
    bias_s = np.zeros((2 * NB, 1), np.float32)
    for h in (0, 1):
        bias_s[h * NB:h * NB + NF, 0] = -kap**3
        bias_s[h * NB + NF, 0] = 1.0       # const row
        bias_s[h * NB + NF + 1, 0] = 0.5   # y + 0.5 = xn >= 0
        bias_s[h * NB + NF + 2, 0] = 0.0   # y^2 >= 0
        bias_s[h * NB + NF + 3, 0] = 0.125  # y^3 + 1/8 >= 0
    # MM2 weights: rows 0-55 = J band, 56 = const col (shift-corrected),
    # 57-59 = A'_1..3; blockdiag
    w2s = np.zeros((NB, NB))
    w2s[:NF, :] = J
    w2s[NF, :] = Ay[0, :] - 0.5 * Ay[1, :] - 0.125 * Ay[3, :]
    w2s[NF + 1, :] = Ay[1, :]
    w2s[NF + 2, :] = Ay[2, :]
    w2s[NF + 3, :] = Ay[3, :]
    w2 = np.zeros((2 * NB, 2 * NB), np.float32)
    w2[:NB, :NB] = w2s
    w2[NB:, NB:] = w2s
    return cu_s, cusT, bias_s, w2, row1


def _numpy_pipeline(x, knots):
    """Host emulation of the device math (f64) — for algebra validation."""
    cu_s, cusT, bias_s, w2, row1 = _make_const_arrays(knots)
    x = np.asarray(x, np.float64).reshape(-1)
    mn, mx = x.min(), x.max()
    inv = 1.0 / (mx - mn + 1e-8)
    a = 16.0 * inv
    bp = -mn * inv - 0.5
    apow = np.array([a, a * a, a ** 3] * 2)[:, None]
    h1 = cu_s[:, 0:2 * NB].astype(np.float64) + \
        bp * cu_s[:, 2 * NB:4 * NB] + bp * bp * cu_s[:, 4 * NB:6 * NB]
    cu_eff = apow * h1
    bias_eff = bias_s[:, 0].astype(np.float64) + \
        bp * cusT[:, 0] + bp**2 * cusT[:, 1] + bp**3 * cusT[:, 2]
    y0 = x / 16.0
    P = np.stack([y0, y0**2, y0**3], 0)          # [3, N]
    half = cu_eff[0:3, 0:NB]                      # same for both halves
    G = half.T @ P + bias_eff[0:NB, None]         # [60, N]
    stack = np.maximum(G, 0.0)
    out = (w2[:NB, :NB].astype(np.float64)).T @ stack  # [60, N]
    return out.T                                   # [N, 60]


# -------------------------------------------------------------- bass program
_CACHE = {}


def _pass_schedule():
    """Static assignment of the 64 PSUM->SBUF passes to DVE/ACT.
    Fixed split (relu on ACT, evict on DVE) keeps each engine's in-order
    queue free of cross-blocking; a couple of iterations hand the evict to
    ACT to rebalance total busy time (ACT 1002ns/pass vs DVE 1146ns)."""
    sched = [('A', 'D')] * NITER
    for i in (11, 21, 28, 31):
        sched[i] = ('A', 'A')
    return sched


def _build_nc():
    import concourse.tile as tile
    from concourse import bacc, mybir

    f32 = mybir.dt.float32
    f32r = mybir.dt.float32r
    bf16 = mybir.dt.bfloat16
    Alu = mybir.AluOpType
    Ax = mybir.AxisListType
    Act = mybir.ActivationFunctionType

    W = TILE_W
    nc = bacc.Bacc("TRN2", target_bir_lowering=False, debug=False)
    xf_d = nc.declare_dram_parameter("xf", [128, XF_COLS], f32, isOutput=False)
    pk_d = nc.declare_dram_parameter("pk", [2 * NB, 4], f32,
                                     isOutput=False)
    w2_d = nc.declare_dram_parameter("w2", [2 * NB, 2 * NB], f32,
                                     isOutput=False)
    cu_s_d = nc.declare_dram_parameter("cu_s", [6, 3 * 2 * NB + 3], f32, isOutput=False)
    out_t = nc.declare_dram_parameter(
        "out_t", [2 * NB, PAIRS * W], bf16, isOutput=True)

    sched = _pass_schedule()

    with tile.TileContext(nc) as tc:
        with (
            tc.tile_pool(name="big", bufs=1) as big_pool,
            tc.tile_pool(name="consts", bufs=1) as const_pool,
            tc.tile_pool(name="work", bufs=1) as work_pool,
            tc.tile_pool(name="stack", bufs=5) as stack_pool,
            tc.tile_pool(name="stage", bufs=6) as stage_pool,
            tc.tile_pool(name="gpsum", bufs=2, space="PSUM") as gpsum_pool,
            tc.tile_pool(name="opsum", bufs=2, space="PSUM") as opsum_pool,
        ):
            # ---- input x (scalar queue): 6 chunks, chunk 0 = own shard.
            # Constants ride the sync queue as two packed DMAs.
            xf = big_pool.tile([128, XF_COLS], f32)
            CHB = [0, W, W + 858, W + 2 * 858, W + 3 * 858, XF_COLS]
            NCH = len(CHB) - 1
            nc.scalar.dma_start(xf[:, 0:W], xf_d[:, 0:W])
            for ci in range(1, NCH):
                nc.scalar.dma_start(
                    xf[:, CHB[ci]:CHB[ci + 1]], xf_d[:, CHB[ci]:CHB[ci + 1]])
            pk = const_pool.tile([2 * NB, 4], f32)
            nc.scalar.dma_start(pk[:, :], pk_d[:, :])
            w2tt = const_pool.tile([2 * NB, 2 * NB], f32r)
            nc.scalar.dma_start(w2tt[:, :], w2_d[:, :].bitcast(f32r))
            cu_s = const_pool.tile([6, 3 * 2 * NB + 3], f32)
            nc.scalar.dma_start(cu_s[:, :], cu_s_d[:, :])
            w2t = w2tt[:, :]
            cusT = pk[:, 0:3]
            bias_s = pk[:, 3:4]
            ones = const_pool.tile([1, 128], f32)
            nc.vector.memset(ones[:, :], 1.0)

            # ---- min/max: DVE does per-chunk min (negated), Pool per-chunk
            # XYZWC max straight to scalars.
            pq = work_pool.tile([128, NCH], f32)      # -min partials
            pmx = work_pool.tile([1, NCH + 1], f32)   # max partials
            # interleave: first chunk's min, then the static power table (so
            # the xr DMAs can start), then the remaining chunk mins.
            nc.vector.tensor_reduce(
                pq[:, 0:1], xf[:, 0:W], Ax.X, Alu.min, negate=True)
            nc.gpsimd.tensor_reduce(
                pmx[0:1, 0:1], xf[:, 0:W], Ax.XYZWC, Alu.max)

            # ---- static power table xp = [y0 | y0^2 | y0^3], y0 = x/16
            # (square on ACT so DVE keeps reducing); xr row pair {q, q+3} =
            # y0^{q+1} of tiles 0-63 / 64-127 — ONE DMA per power via a
            # partition-strided destination AP.
            xp0 = big_pool.tile([128, W], f32)
            xp1 = big_pool.tile([128, W], f32)
            xp2 = big_pool.tile([128, W], f32)
            xp = [xp0, xp1, xp2]
            xr = big_pool.tile([6, PAIRS * W], f32r)

            def xr_dma(q):
                nc.sync.dma_start(
                    xr[q:q + 4:3, :].rearrange("p (j c) -> p j c", c=W),
                    xp[q][:, :].bitcast(f32r))

            with tc.high_priority():
                nc.vector.tensor_scalar(
                    xp[0][:, :], xf[:, 0:W], 0.0, 0.0625,
                    Alu.subtract, Alu.mult)
                xr_dma(0)
                nc.vector.tensor_mul(xp[1][:, :], xp[0][:, :], xp[0][:, :])
                xr_dma(1)
                nc.vector.tensor_mul(xp[2][:, :], xp[1][:, :], xp[0][:, :])
                xr_dma(2)

            for ci in range(1, NCH):
                nc.vector.tensor_reduce(
                    pq[:, ci:ci + 1], xf[:, CHB[ci]:CHB[ci + 1]],
                    Ax.X, Alu.min, negate=True)
                nc.gpsimd.tensor_reduce(
                    pmx[0:1, ci:ci + 1], xf[:, CHB[ci]:CHB[ci + 1]],
                    Ax.XYZWC, Alu.max)

            # ---- combine to scalars g = [-min | max]
            pm = work_pool.tile([128, 1], f32)
            nc.vector.tensor_reduce(pm[:, 0:1], pq[:, :], Ax.X, Alu.max)
            g = work_pool.tile([1, 2], f32)
            nc.gpsimd.tensor_reduce(g[0:1, 0:1], pm[:, 0:1], Ax.XYZWC, Alu.max)
            nc.gpsimd.tensor_reduce(g[0:1, 1:2], pmx[0:1, 0:NCH], Ax.XYZWC,
                                    Alu.max)

            # ---- broadcast (-min, max) to all partitions, then the scalar
            # chain runs on [128,1] columns of S.
            #   S cols: 0 3ab'^2 | 1 3a^2b' | 2 a^3 | 3 2ab' | 4 a^2 | 5 a
            #           | 6 b' | 7 b'^2 | 8 b'^3 | 9 span | 10 inv | 11 ab'
            bc = opsum_pool.tile([128, 2], f32, tag="op")
            nc.tensor.matmul(bc[:, :], ones[:, :], g[0:1, 0:2])
            S = work_pool.tile([128, 12], f32)
            v = nc.vector
            # span = max + (-min) + 1e-8 ; inv = 1/span
            v.tensor_scalar(S[:, 9:10], bc[:, 1:2], bc[:, 0:1], 1e-8,
                            Alu.add, Alu.add)
            v.reciprocal(S[:, 10:11], S[:, 9:10])
            # a = 16*inv ; b' = (-min)*inv - 0.5
            v.tensor_scalar_mul(S[:, 5:6], S[:, 10:11], 16.0)
            v.tensor_scalar(S[:, 6:7], S[:, 10:11], bc[:, 0:1], -0.5,
                            Alu.mult, Alu.add)
            v.tensor_mul(S[:, 4:5], S[:, 5:6], S[:, 5:6])          # a^2
            v.tensor_mul(S[:, 2:3], S[:, 4:5], S[:, 5:6])          # a^3
            v.tensor_mul(S[:, 7:8], S[:, 6:7], S[:, 6:7])          # b'^2
            v.tensor_mul(S[:, 8:9], S[:, 7:8], S[:, 6:7])          # b'^3

            # ---- MM1 weight composition (Horner in b', then scale by
            # the per-partition a-power column)
            apow = work_pool.tile([6, 1], f32)
            v.tensor_scalar(apow[:, :], cu_s[:, 6 * NB:6 * NB + 1],
                            S[0:6, 5:6], None, Alu.mult)
            v.scalar_tensor_tensor(apow[:, :], cu_s[:, 6 * NB + 1:6 * NB + 2],
                                   S[0:6, 4:5], apow[:, :],
                                   Alu.mult, Alu.add)
            v.scalar_tensor_tensor(apow[:, :], cu_s[:, 6 * NB + 2:6 * NB + 3],
                                   S[0:6, 2:3], apow[:, :],
                                   Alu.mult, Alu.add)
            h1 = work_pool.tile([6, 2 * NB], f32)
            v.scalar_tensor_tensor(h1[:, :], cu_s[:, 2 * NB:4 * NB],
                                   S[0:6, 6:7], cu_s[:, 0:2 * NB],
                                   Alu.mult, Alu.add)
            v.scalar_tensor_tensor(h1[:, :], cu_s[:, 4 * NB:6 * NB],
                                   S[0:6, 7:8], h1[:, :],
                                   Alu.mult, Alu.add)
            cu_eff = work_pool.tile([6, 2 * NB], f32r)
            v.tensor_scalar(cu_eff[:, :], h1[:, :], apow[:, 0:1], None,
                            Alu.mult)

            # ---- bias composition on Pool: bias_eff = bias_s + b'*cusT0
            # + b'^2*cusT1 + b'^3*cusT2
            t1 = work_pool.tile([2 * NB, 1], f32)
            t2 = work_pool.tile([2 * NB, 1], f32)
            bias_eff = work_pool.tile([2 * NB, 1], f32)
            v.scalar_tensor_tensor(t1[:, :], cusT[:, 0:1], S[0:120, 6:7],
                                    bias_s[:, :], Alu.mult, Alu.add)
            v.scalar_tensor_tensor(t2[:, :], cusT[:, 1:2], S[0:120, 7:8],
                                   t1[:, :], Alu.mult, Alu.add)
            v.scalar_tensor_tensor(bias_eff[:, :], cusT[:, 2:3],
                                   S[0:120, 8:9], t2[:, :],
                                   Alu.mult, Alu.add)

            # ---- main pipeline, software-pipelined so the in-order tensor
            # engine never parks behind a sem-waiting MM2: issue order is
            # MM1(i) | relu(i) ... MM1(i+1) | MM2(i) | evict(i) | out(i).
            BANK = 512
            cu_r = cu_eff[:, :]
            gps = [None] * NITER
            stks = [None] * NITER

            def xr_slice(pb):
                return xr[0:6, pb * W:(pb + 1) * W]

            def issue_mm1(i):
                gp = gpsum_pool.tile([120, 2 * BANK], f32, tag="gp")
                gps[i] = gp
                nc.tensor.matmul(gp[:, 0:W], cu_r, xr_slice(2 * i))
                nc.tensor.matmul(gp[:, BANK:BANK + W], cu_r,
                                 xr_slice(2 * i + 1))

            def issue_relu(i):
                gp = gps[i]
                stk = stack_pool.tile([120, 2 * W], f32r)
                stks[i] = stk
                gp_view = gp[:, :].rearrange(
                    "r (p c) -> r p c", c=BANK)[:, :, 0:W]
                stk_view = stk[:, :].rearrange("r (p c) -> r p c", c=W)
                if sched[i][0] == 'D':
                    nc.vector.tensor_scalar(
                        stk_view, gp_view, bias_eff[:, 0:1], 0.0,
                        Alu.add, Alu.max)
                else:
                    nc.scalar.activation(
                        stk_view, gp_view, Act.Relu,
                        bias=bias_eff[:, 0:1], scale=1.0)

            def issue_tail(i):
                stk = stks[i]
                op = opsum_pool.tile([120, 2 * BANK], f32, tag="op")
                nc.tensor.matmul(op[:, 0:W], w2t,
                                 stk[:, 0:W])
                nc.tensor.matmul(op[:, BANK:BANK + W], w2t,
                                 stk[:, W:2 * W])
                stg = stage_pool.tile([120, 2 * W], bf16)
                op_view = op[:, :].rearrange(
                    "r (p c) -> r p c", c=BANK)[:, :, 0:W]
                stg_view = stg[:, :].rearrange("r (p c) -> r p c", c=W)
                if sched[i][1] == 'D':
                    nc.vector.tensor_scalar_add(stg_view, op_view, 0.0)
                    nc.sync.dma_start(
                        out_t[:, i * 2 * W:(i + 1) * 2 * W], stg[:, :])
                elif sched[i][1] == 'S':
                    # split evict: separate tiles so the two engines do not
                    # share a tile (tile-granular deps would serialize them)
                    stgb = stage_pool.tile([120, W], bf16)
                    nc.vector.tensor_scalar_add(stg[:, 0:W], op[:, 0:W], 0.0)
                    nc.scalar.activation(stgb[:, :], op[:, BANK:BANK + W],
                                         Act.Copy)
                    nc.sync.dma_start(
                        out_t[:, i * 2 * W:i * 2 * W + W], stg[:, 0:W])
                    nc.sync.dma_start(
                        out_t[:, i * 2 * W + W:(i + 1) * 2 * W], stgb[:, :])
                else:
                    nc.scalar.activation(stg_view, op_view, Act.Copy)
                    nc.sync.dma_start(
                        out_t[:, i * 2 * W:(i + 1) * 2 * W], stg[:, :])

            issue_mm1(0)
            issue_relu(0)
            for i in range(1, NITER):
                issue_mm1(i)
                if sched[i - 1] == ('A', 'A'):
                    # swap iteration: let the next relu go first in ACT's
                    # queue so the extra evict doesn't starve DVE
                    issue_relu(i)
                    issue_tail(i - 1)
                else:
                    issue_tail(i - 1)
                    issue_relu(i)
            issue_tail(NITER - 1)

    nc.compile()
    return nc


# ------------------------------------------------------------------- driver
def _run(in_maps, trace=False):
    from concourse.bass_utils import run_bass_kernel_spmd

    if "nc" not in _CACHE:
        _CACHE["nc"] = _build_nc()
    return run_bass_kernel_spmd(
        _CACHE["nc"], in_maps, list(range(N_CORES)), trace=trace
    )


def _default_knots():
    inner = np.linspace(0.0, 1.0, NUM_KNOTS - 2 * DEGREE)
    return np.concatenate(
        [np.zeros(DEGREE), inner, np.ones(DEGREE)]
    ).astype(np.float32)


def kernel(x, knots=None, degree=None, _trace=False, _return_results=False, **_):
    x = np.asarray(x, np.float32).reshape(-1)
    assert x.size == N_POINTS
    if knots is None:
        knots = _default_knots()
    cu_s, cusT, bias_s, w2, row1 = _make_const_arrays(
        np.asarray(knots, np.float64))

    # per-shard padded tiles [8][128, 490]
    tiles = []
    for c in range(N_CORES):
        sh = np.empty(SHARD_PAD, np.float32)
        sh[:SHARD] = x[c * SHARD:(c + 1) * SHARD]
        sh[SHARD:] = x[c * SHARD:c * SHARD + SHARD_PAD - SHARD]
        tiles.append(sh.reshape(128, TILE_W))

    pk = np.zeros((2 * NB, 4), np.float32)
    pk[:, 0:3] = cusT
    pk[:, 3:4] = bias_s
    in_maps = []
    for c in range(N_CORES):
        xf = np.concatenate([tiles[(c + k) % N_CORES] for k in range(N_CORES)],
                            axis=1)
        in_maps.append({
            "xf": np.ascontiguousarray(xf),
            "pk": pk,
            "w2": w2,
            "cu_s": cu_s,
        })

    res = _run(in_maps, trace=_trace)

    out = np.empty((N_POINTS, NB), np.float32)
    for c in range(N_CORES):
        ot = np.asarray(res.results[c]["out_t"]).astype(np.float32)
        # rows (h, j), cols (pb, c) -> point (h*64+pb)*490+c, output j
        full = ot.reshape(2, NB, PAIRS, TILE_W).transpose(0, 2, 3, 1).reshape(
            SHARD_PAD, NB)
        out[c * SHARD:(c + 1) * SHARD, :] = full[:SHARD]

    # boundary fixup: at xn == 1.0 exactly the reference jumps to the
    # degenerate right-end pieces (s = 64); patch those rows exactly
    mn, mx = x.min(), x.max()
    xn = (x - mn) / ((mx - mn) + np.float32(1e-8))
    at_one = np.nonzero(xn == np.float32(1.0))[0]
    if at_one.size:
        out[at_one, :] = row1.astype(np.float32)[None, :]

    if _return_results:
        return out, res
    return out
